# revision 4
# baseline (speedup 1.0000x reference)
"""BilinearInteraction Trainium2 kernel (8 NeuronCores, batch-sharded).

out[b, p=(i,j), d] = x[b, i, d] * (x @ W)[b, j, d]  for the 496 upper-tri
pairs of F=32 fields; x [4096, 32, 64] f32, W [64, 64] f32.

bf16 end-to-end (harness gate is rel_err < 2e-2; this pipeline lands at
~5.5e-3): DVE tensor_tensor runs in 2x_1P mode and the HBM store traffic
halves vs f32. The kernel is DVE-bound, so the remaining structure is
aimed at DVE instruction count:

  - vid = x @ W on PE (pair-block transposes + bf16 matmuls against a
    block-diag [[W,0],[0,W]]), landing in PSUM f32.
  - ACT copies vid PSUM->SBUF TWICE: plane 0 = vid[f], plane 1 = vid[f+1]
    (shifted by one field). This lets one DVE tensor_mul cover TWO
    adjacent pair-blocks (i, i+1) with a single affine 4D access pattern
    vd[:, 0:2, i+1:i+1+nj, :], halving the per-instruction fixed cost
    (~150 ns x 31 -> x 16 per tile). Block i+1 is padded to block i's
    width; the one garbage slot per merged op sits at the end of the
    staging tile and is simply not stored.
  - merged ops run in descending i (small ops first) so the first store
    fires ~5 us into the kernel; each op's staging tile is DMA'd as one
    contiguous-per-partition store on the sync HWDGE ring; inputs ride
    the scalar-engine ring so they never queue behind output stores.
  - tile 0's x loads high-fields-first so the PE/DVE pipeline starts
    after half a tile load.
Host converts x/W to bf16 on the way in, result back to f32 on the out.
"""

import sys

if "/opt/trn_rl_repo" not in sys.path:
    sys.path.insert(0, "/opt/trn_rl_repo")

import numpy as np
import ml_dtypes

import concourse.bass as bass
import concourse.mybir as mybir
import concourse.tile as tile
from concourse import bacc
from concourse.bass_utils import run_bass_kernel_spmd

B, F, D = 4096, 32, 64
P = F * (F - 1) // 2  # 496
NCORES = 8
BSH = B // NCORES  # 512 batch rows per core
BT = 128  # batch tile (SBUF partitions)
NTILES = BSH // BT  # 4
FD = F * D  # 2048

bf16 = mybir.dt.bfloat16
f32 = mybir.dt.float32
np_bf16 = ml_dtypes.bfloat16

# pair-block offsets: block i = pairs (i, j) for j in i+1..F-1
POFF = [0]
for i in range(F - 1):
    POFF.append(POFF[-1] + (F - 1 - i))

# merged DVE ops, descending i: (i0, m, nj) covers blocks i0..i0+m-1,
# each padded to nj=F-1-i0 rows; valid output = m*nj-(m-1) contiguous
# pairs at POFF[i0]
MOPS = [(F - 2, 1, 1)] + [(i, 2, F - 1 - i) for i in range(F - 4, -1, -2)]


def _emit(tc, nc, x_d, w2_d, i128_d, out_d):
    with (
        tc.tile_pool(name="const", bufs=1) as const_pool,
        tc.tile_pool(name="xp", bufs=4) as x_pool,
        tc.tile_pool(name="vidp", bufs=2) as vid_pool,
        tc.tile_pool(name="xtp", bufs=4) as xt_pool,
        tc.tile_pool(name="outp", bufs=8) as out_pool,
        tc.tile_pool(name="ps_t", bufs=2, space="PSUM") as ps_t,
        tc.tile_pool(name="ps_m", bufs=2, space="PSUM") as ps_m,
    ):
        # inputs ride the scalar-engine HWDGE ring, constants first;
        # outputs own the sync HWDGE ring (a shared FIFO would park tile
        # t+1's x load behind tile t's output stores and starve the DVE).
        x_ts = []
        for t in range(NTILES):
            x_t = x_pool.tile([128, FD], bf16, tag="xt")
            x_ts.append(x_t)
        # tile 0 loads high fields first: the first-processed merged ops
        # only read x fields >=16 and vid groups 3,2 (fields >=16), so
        # the PE/DVE pipeline starts after half a tile load.
        nc.scalar.dma_start(
            out=x_ts[0][:, FD // 2 :].rearrange("p (f d) -> p f d", d=D),
            in_=x_d[0:BT, F // 2 :, :],
        )
        ident = const_pool.tile([128, 128], bf16)
        nc.scalar.dma_start(out=ident[:], in_=i128_d[:])
        w2 = const_pool.tile([128, 128], bf16)
        nc.scalar.dma_start(out=w2[:], in_=w2_d[:])
        nc.scalar.dma_start(
            out=x_ts[0][:, : FD // 2].rearrange("p (f d) -> p f d", d=D),
            in_=x_d[0:BT, : F // 2, :],
        )
        for t in range(1, NTILES):
            nc.scalar.dma_start(
                out=x_ts[t][:].rearrange("p (f d) -> p f d", d=D),
                in_=x_d[t * BT : (t + 1) * BT, :, :],
            )

        for t in range(NTILES):
            b0 = t * BT
            x_t = x_ts[t]
            x3 = x_t[:].rearrange("p (f d) -> p f d", d=D)

            # vid in 4 descending groups of 4 f-pairs (one PSUM bank each):
            # 4 transposes + 1 ACT copy + 4 matmuls + 2 ACT copies per
            # group (plane 0 unshifted, plane 1 shifted one field down).
            vid_t = vid_pool.tile([128, 2 * FD], bf16, tag="vidt")
            for g in reversed(range(4)):
                xT_ps = ps_t.tile([128, 512], bf16, tag="xtps")
                for k in range(4):
                    nc.tensor.transpose(
                        xT_ps[:, k * 128 : (k + 1) * 128],
                        x_t[:, (4 * g + k) * 128 : (4 * g + k + 1) * 128],
                        ident[:],
                    )
                xT_sb = xt_pool.tile([128, 512], bf16, tag="xtsb")
                nc.scalar.copy(xT_sb[:], xT_ps[:])
                vid_ps = ps_m.tile([128, 512], f32, tag="vidps")
                for k in range(4):
                    nc.tensor.matmul(
                        vid_ps[:, k * 128 : (k + 1) * 128],
                        xT_sb[:, k * 128 : (k + 1) * 128],
                        w2[:],
                        start=True,
                        stop=True,
                    )
                # plane 0: fields 8g..8g+7 at [g*512, (g+1)*512)
                nc.scalar.copy(vid_t[:, g * 512 : (g + 1) * 512], vid_ps[:])
                # plane 1: dup1[f-1] = vid[f] (field 0 has no slot)
                if g == 0:
                    nc.scalar.copy(
                        vid_t[:, FD : FD + 7 * D], vid_ps[:, D:512]
                    )
                else:
                    nc.scalar.copy(
                        vid_t[:, FD + (8 * g - 1) * D : FD + (8 * g + 7) * D],
                        vid_ps[:],
                    )
                if g == 3:
                    # dup1[31] backs the (never-stored) garbage slot of
                    # each merged op; any defined value works
                    nc.scalar.copy(
                        vid_t[:, FD + 31 * D : FD + 32 * D], vid_ps[:, 448:512]
                    )
            # [128, plane, field, d]
            vd = vid_t[:].rearrange("p (u f d) -> p u f d", u=2, d=D)

            for i0, m, nj in MOPS:
                o_t = out_pool.tile([128, m * nj * D], bf16, tag="outs")
                o4 = o_t[:].rearrange("p (u q d) -> p u q d", u=m, d=D)
                in0 = (
                    x3[:, i0 : i0 + m, :]
                    .unsqueeze(2)
                    .broadcast_to((128, m, nj, D))
                )
                in1 = vd[:, 0:m, i0 + 1 : i0 + 1 + nj, :]
                nc.vector.tensor_mul(o4, in0, in1)
                valid = m * nj - (m - 1)
                o3 = o_t[:].rearrange("p (q d) -> p q d", d=D)
                nc.sync.dma_start(
                    out=out_d[b0 : b0 + BT, POFF[i0] : POFF[i0] + valid, :],
                    in_=o3[:, 0:valid, :],
                )


def build_nc():
    nc = bacc.Bacc("TRN2", target_bir_lowering=False, debug=False)
    x_d = nc.dram_tensor("x", [BSH, F, D], bf16, kind="ExternalInput")
    w2_d = nc.dram_tensor("W2", [128, 128], bf16, kind="ExternalInput")
    i128_d = nc.dram_tensor("I128", [128, 128], bf16, kind="ExternalInput")
    out_d = nc.dram_tensor("out", [BSH, P, D], bf16, kind="ExternalOutput")
    with tile.TileContext(nc) as tc:
        _emit(tc, nc, x_d.ap(), w2_d.ap(), i128_d.ap(), out_d.ap())
    nc.compile()
    return nc


_NC = None


def kernel(x: np.ndarray, W: np.ndarray, _trace=False, _trace_kwargs=None):
    global _NC
    if _NC is None:
        _NC = build_nc()
    x16 = np.ascontiguousarray(x, dtype=np.float32).astype(np_bf16)
    W = np.ascontiguousarray(W, dtype=np.float32)
    w2 = np.zeros((128, 128), dtype=np.float32)
    w2[:64, :64] = W
    w2[64:, 64:] = W
    w2 = w2.astype(np_bf16)
    i128 = np.eye(128, dtype=np_bf16)
    in_maps = [
        {"x": x16[i * BSH : (i + 1) * BSH], "W2": w2, "I128": i128}
        for i in range(NCORES)
    ]
    res = run_bass_kernel_spmd(
        _NC,
        in_maps,
        core_ids=list(range(NCORES)),
        trace=_trace,
        **(_trace_kwargs or {}),
    )
    out = np.concatenate(
        [res.results[i]["out"].astype(np.float32) for i in range(NCORES)], axis=0
    )
    if _trace:
        return out, res
    return out
